# revision 1
# baseline (speedup 1.0000x reference)
"""FP8-quantized dense MLP (scaled matmul) on 8 Trainium2 NeuronCores.

Reference computation:
    x  [8, 2048, 4096] f32, weight [4096, 4096] f32
    sx = 448 / amax(|x|); sw = 448 / amax(|w|)
    out = (q8(x*sx) @ q8(w*sw)) * (1/sx) * (1/sw)     (q8 = OCP e4m3fn RNE)

Sharding: 4 M-shards x 2 N-shards over 8 cores (core c -> rows
[c//2*4096, +4096), cols [c%2*2048, +2048)).  Scales + fp8 quantization run
on host (O(MK+KN) elementwise prep); the O(MKN) matmul runs on device.

TRN2's FP8_EXP4 has max +-240 (OCP e4m3fn has +-448), so OCP-quantized values
256..448 would be NaN/Inf on device.  We therefore quantize to the OCP grid
*halved* (exact in fp8 for all but deep-subnormal values) by scaling with
sx/2 and clipping to +-224, and compensate with a *4 factor folded into the
output scale.  The device matmul (fp8 products, f32 accumulate) is then
bit-equivalent to the reference modulo f32 summation order.

Device kernel per core: out[4096, 2048] = xT.T @ w in fp8 DoubleRow mode
(K-tiles of 256).  The PE stream runs at the hardware fp8 pitch (216ns per
128x512xK256 matmul = 512 cols at the effective ~2.37GHz clock; ~442us for
the 2048 matmuls), so the schedule optimizes the edges:

- Prologue: framework preamble owns all engines until ~6.1us; first DMA
  bytes land ~8.4us and the early DMA path is DESCRIPTOR-rate limited
  (~35-55 desc/us cold, one descriptor per partition per contiguous run),
  so x0 rides the scalar HW-DGE queue concurrently with w0 leading the
  sync queue (whole tiles only -- sub-tile splits multiply descriptors).
  Warm-up matmuls (HAM clock ramp: 1.2GHz until ~3.4us busy, reset by
  >=1.5us idle) keep the PE hot until the first data-gated matmul fires,
  plus a batch parked between the k0 and k1 groups where the still-ramping
  weight stream would otherwise stall the PE.
- m-tiles 0+1 run interleaved across k2 on ALL 8 PSUM banks, m1 skewed one
  k2 behind m0 (x1 arrives after w0/w1), so one arriving 524KB weight tile
  feeds 8 matmuls (~1.7us) -- a full-speed PE never outruns the weight
  stream (~1.25us/tile warm).  The warm-up tile shares bank b7 by tag with
  m1's n3 accumulator; b4-b7 are allocated only after the last warm-up
  write, because allocating a same-tag tile ROTATES the (bufs=1) slot and
  writing the rotated-out tile races the new owner (PSUM collision, device
  crash).
- m-tiles 2..30: 4 banks, alternating bank sets (b0-3 / b4-7) per m-tile,
  k2-inner; evictions (scalar/vector alternating, scaled copy) write one
  [128, 2048] SBUF tile per m-tile and a SINGLE out-DMA moves it (a DMA
  trigger costs ~0.6us of engine time; 1 trigger instead of 4).
- Last m-tile is n-outer (16 k2 matmuls per bank, then evict+DMA that bank
  immediately); the final bank evicts by partition-half on both engines in
  parallel, each half's 128KB DMA (sync/scalar) launching as its half
  lands.  Dummy reads between the per-bank out-DMAs lean against the
  DMA path going descriptor-cold before the final drain.
"""

import numpy as np
import ml_dtypes

FP8_MAX = 448.0
B, S, K, N = 8, 2048, 4096, 4096
NCORES = 8
MSHARDS = 4
NSHARDS = 2
M_CORE = B * S // MSHARDS   # 4096 rows per core
N_CORE = N // NSHARDS       # 2048 cols per core
P = 128
KS = K // P      # 32 k-subtiles of 128 (partition dim)
K2 = K // 256    # 16 DoubleRow k-tiles of 256
MT = M_CORE // P  # 32 m-tiles per core
NFREE = 512      # matmul free dim == one PSUM bank of f32
NT = N_CORE // NFREE  # 4 PSUM banks per m-tile

WARMUPS = 46     # PE warm-up matmuls (see _build_nc)
MID_WARMUPS = 16  # warm-ups parked between the k2=0 and k2=1 groups:
# sized so a slow-ramp run's w1 lateness (up to ~2.1us beyond typical)
# is absorbed as warm-up work instead of a >=1.5us idle that resets the
# PE clock to 1.2GHz (a ~3us tax when it happens)

_E4M3 = ml_dtypes.float8_e4m3  # TRN semantics: max +-240

_nc_cache = None


def _build_nc():
    from concourse import bacc, tile, mybir

    nc = bacc.Bacc("TRN2", debug=False)
    xt_d = nc.dram_tensor("xt", [MT, P, KS, P], mybir.dt.float8e4, kind="ExternalInput")
    wt_d = nc.dram_tensor(
        "wt", [K2, P, 2, N_CORE], mybir.dt.float8e4, kind="ExternalInput"
    )
    sc_d = nc.dram_tensor("sc", [P, 1], mybir.dt.float32, kind="ExternalInput")
    out_d = nc.dram_tensor("out", [M_CORE, N_CORE], mybir.dt.float32, kind="ExternalOutput")

    DR = mybir.MatmulPerfMode.DoubleRow
    KH = KS // 2   # x half-tile boundary (k2 0..7 | 8..15)
    NH = N_CORE // 2

    with tile.TileContext(nc) as tc:
        with (
            tc.tile_pool(name="wp", bufs=1) as wp,
            tc.tile_pool(name="xp", bufs=4) as xp,
            tc.tile_pool(name="op", bufs=3) as op,
            tc.tile_pool(name="fp", bufs=2) as fpool,
            tc.tile_pool(name="cp", bufs=1) as cp,
            tc.tile_pool(name="pp", bufs=1, space="PSUM") as pp,
        ):
            # PE warm-up: the HAM clock gate keeps the PE at 1.2 GHz until it
            # has been busy ~3.4us; an idle gap resets the ramp.  Dummy
            # matmuls on a zeroed scratch tile keep the PE busy from the end
            # of the framework preamble until the first data lands, so the
            # real stream starts at 2.4 GHz.  One memset only (wa serves as
            # both operands) so warm-ups start as early as possible.  The
            # warm-up accumulator shares bank b7 by tag (see module doc).
            wa = cp.tile([P, 2, P], mybir.dt.float8e4, tag="wa")
            nc.vector.memset(wa[:], 0)
            psw = pp.tile([P, P], mybir.dt.float32, tag="b7", bufs=1, name="psw")
            for _ in range(WARMUPS):
                nc.tensor.matmul(
                    psw[:], wa[:], wa[:], start=True, stop=True, perf_mode=DR
                )

            sc_sb = cp.tile([P, 1], mybir.dt.float32, tag="sc")

            x_first = {
                m: xp.tile([P, KS, P], mybir.dt.float8e4, tag="x", name=f"x{m}")
                for m in (0, 1)
            }
            w_sb = [
                wp.tile([P, 2, N_CORE], mybir.dt.float8e4, tag=f"w{k2}",
                        name=f"w{k2}")
                for k2 in range(K2)
            ]

            # The early DMA system is DESCRIPTOR-rate limited (~50-100
            # descriptors/us while it ramps, ~105/us warm); every transfer
            # costs one descriptor per partition per contiguous run, so
            # sub-tile splits only add descriptors (halving w0 by columns
            # quadruples its count: 4KB rows -> 4x 1KB fragments).  Whole
            # tiles only.  The two HW-DGE queues ramp independently, so x0/x1
            # (+ the tiny sc, whose [128,1] layout is 128 4-byte descriptors
            # that would stall the weight stream) ride the scalar queue while
            # the sync queue carries nothing but the 16 weight tiles -- the
            # first matmul's inputs arrive on both queues concurrently.
            # gpsimd's software-DGE queue measured a 95us regression in a
            # previous session.
            # Queue-priming and all-on-one-queue arrangements were both
            # measured NO better: the two HW-DGE queues COMPETE for one
            # cold-ramping descriptor-engine pool (~45-55 desc/us shared), so
            # the first matmul's 256 descriptors (x0 + w0) cost ~4-5us after
            # the ~8.4us data start no matter how they are distributed.
            # x0 rides the scalar queue so the sync queue is a pure weight
            # stream; sc (128 4-byte descriptors) stays off it too.
            nc.scalar.dma_start(x_first[0][:], xt_d[0])
            nc.scalar.dma_start(sc_sb[:], sc_d[:])
            for k2 in (0, 1, 2, 3):
                nc.sync.dma_start(w_sb[k2][:], wt_d[k2])
            nc.sync.dma_start(x_first[1][:], xt_d[1])
            for k2 in range(4, K2):
                nc.sync.dma_start(w_sb[k2][:], wt_d[k2])

            def mm_one(bank, x_t, k2, n, start, stop, load=True):
                inst = nc.tensor.matmul(
                    bank[:],
                    x_t[:, 2 * k2 : 2 * k2 + 2, :],
                    w_sb[k2][:, :, n * NFREE : (n + 1) * NFREE],
                    start=start,
                    stop=stop,
                    perf_mode=DR,
                )
                if not load:
                    # The 4 n-chunks of a k2 group share the same stationary
                    # (the x slice).  Hint codegen to skip the redundant
                    # LDWEIGHTS; measured: current walrus IGNORES this flag
                    # (still 1 LDWEIGHTS per matmul, pitch 216ns either way),
                    # kept as documentation of the attempt.
                    inst.ins.ldweights = False

            def evict_mtile(m, banks):
                o_t = op.tile([P, N_CORE], mybir.dt.float32, tag="o", name=f"o{m}")
                for n in range(NT):
                    dst = o_t[:, n * NFREE : (n + 1) * NFREE]
                    if n % 2 == 0:
                        nc.scalar.activation(
                            dst,
                            banks[n][:],
                            mybir.ActivationFunctionType.Copy,
                            scale=sc_sb[:],
                        )
                    else:
                        nc.vector.tensor_scalar_mul(dst, banks[n][:], sc_sb[:])
                nc.sync.dma_start(out_d[m * P : (m + 1) * P, :], o_t[:])

            # ---- m-tiles 0+1, interleaved across k2 on all 8 banks ----
            # Skewed by one k2 (m1 lags m0) so m1's first matmul lands when
            # x1 -- second on the scalar queue -- has arrived; each w tile
            # still feeds 8 matmuls (~1.7us) per ~1.25us arrival, so a
            # full-speed PE never outruns the weight stream.  PSUM
            # accumulation order within a bank is k2-agnostic.
            b01 = [
                pp.tile([P, NFREE], mybir.dt.float32, tag=f"b{i}", bufs=1,
                        name=f"ps01_{i}")
                for i in range(4)
            ]
            for n in range(NT):
                mm_one(b01[n], x_first[0], 0, n, True, False, load=n == 0)
            # The k2=0 group outruns the still-ramping weight stream; park
            # the PE on warm-up matmuls instead of idling.  m1's banks b4-b7
            # are deliberately allocated only AFTER these writes: allocating
            # a same-tag tile rotates the (bufs=1) slot, and writing the
            # rotated-out psw tile afterwards races the new owner's
            # accumulation (PSUM_COLLISION device crash).
            for _ in range(MID_WARMUPS):
                nc.tensor.matmul(
                    psw[:], wa[:], wa[:], start=True, stop=True, perf_mode=DR
                )
            b01 += [
                pp.tile([P, NFREE], mybir.dt.float32, tag=f"b{i}", bufs=1,
                        name=f"ps01_{i}")
                for i in range(4, 8)
            ]
            # m1 lags m0 by SKEW k2-steps: blocks k1..k(SKEW-1) are m0-only
            # (0.85us demand per w tile), so a lagging cold-ramp weight
            # stream produces several sub-1.5us stalls (no clock reset)
            # instead of one big resetting one; m1 repays the lag in
            # supply-free tail blocks.  m0 finishes first, so its eviction
            # overlaps m1's tail.
            SKEW = 4
            for k2 in range(1, K2):
                for n in range(NT):
                    mm_one(b01[n], x_first[0], k2, n, False, k2 == K2 - 1,
                           load=n == 0)
                if k2 >= SKEW:
                    j = k2 - SKEW
                    for n in range(NT):
                        mm_one(b01[4 + n], x_first[1], j, n, j == 0, False,
                               load=n == 0)
            evict_mtile(0, b01[0:4])
            for j in range(K2 - SKEW, K2):
                for n in range(NT):
                    mm_one(b01[4 + n], x_first[1], j, n, False, j == K2 - 1,
                           load=n == 0)
            evict_mtile(1, b01[4:8])

            # ---- m-tiles 2..30: 4 banks, alternating sets, k2-inner ----
            for m in range(2, MT - 1):
                x_t = xp.tile([P, KS, P], mybir.dt.float8e4, tag="x", name=f"x{m}")
                nc.sync.dma_start(x_t[:], xt_d[m])
                base = (m % 2) * 4
                banks = [
                    pp.tile([P, NFREE], mybir.dt.float32, tag=f"b{base + n}",
                            bufs=1, name=f"ps{m}_{n}")
                    for n in range(NT)
                ]
                for k2 in range(K2):
                    for n in range(NT):
                        mm_one(banks[n], x_t, k2, n, k2 == 0, k2 == K2 - 1,
                               load=n == 0)
                evict_mtile(m, banks)

            # ---- last m-tile: n-outer so the tail is one bank deep.  The
            # DMA system goes descriptor-cold (~45 desc/us) within a few us
            # of idling, so dummy reads between the per-bank out-DMAs keep it
            # warm, the final bank's eviction is split across both engines,
            # and its DMA into four 32-partition pieces across both queues.
            m = MT - 1
            x_t = xp.tile([P, KS, P], mybir.dt.float8e4, tag="x", name=f"x{m}")
            nc.sync.dma_start(x_t[:], xt_d[m])
            for n in range(NT):
                bank = pp.tile([P, NFREE], mybir.dt.float32, tag=f"b{4 + n}",
                               bufs=1, name=f"ps{m}_{n}")
                for k2 in range(K2):
                    mm_one(bank, x_t, k2, n, k2 == 0, k2 == K2 - 1)
                if n < NT - 1:
                    o_t = fpool.tile([P, NFREE], mybir.dt.float32, tag="of",
                                     name=f"o{m}_{n}")
                    if n % 2 == 0:
                        nc.scalar.activation(
                            o_t[:], bank[:], mybir.ActivationFunctionType.Copy,
                            scale=sc_sb[:],
                        )
                    else:
                        nc.vector.tensor_scalar_mul(o_t[:], bank[:], sc_sb[:])
                    nc.sync.dma_start(
                        out_d[m * P : (m + 1) * P,
                              n * NFREE : (n + 1) * NFREE],
                        o_t[:],
                    )
                else:
                    # final bank: evict by partition-half on both engines and
                    # launch each half's DMA as soon as its half lands.  The
                    # halves go to SEPARATE tiles: sharing one tile serialized
                    # them (subtile hazard tracking does not split partition
                    # ranges -- measured: the vector copy waited on the
                    # scalar copy's completion semaphore).
                    hp = P // 2
                    o_lo = fpool.tile([P, NFREE], mybir.dt.float32,
                                      tag="oflo", name=f"o{m}_{n}lo")
                    o_hi = fpool.tile([P, NFREE], mybir.dt.float32,
                                      tag="ofhi", name=f"o{m}_{n}hi")
                    nc.scalar.activation(
                        o_lo[0:hp, :], bank[0:hp, :],
                        mybir.ActivationFunctionType.Copy,
                        scale=sc_sb[0:hp, :],
                    )
                    nc.sync.dma_start(
                        out_d[m * P : m * P + hp,
                              n * NFREE : (n + 1) * NFREE],
                        o_lo[0:hp, :],
                    )
                    nc.vector.tensor_scalar_mul(
                        o_hi[hp:, :], bank[hp:, :], sc_sb[hp:, :]
                    )
                    nc.scalar.dma_start(
                        out_d[m * P + hp : (m + 1) * P,
                              n * NFREE : (n + 1) * NFREE],
                        o_hi[hp:, :],
                    )

    nc.finalize()
    return nc


def _get_nc():
    global _nc_cache
    if _nc_cache is None:
        _nc_cache = _build_nc()
    return _nc_cache


def _amax(a):
    # max(|a|) without a full |a| temp; exact (max/min are exact in f32)
    return np.float32(max(np.float32(a.max()), -np.float32(a.min())))


def _prep(x, weight):
    """Host prep: scales, halved OCP-grid fp8 quantization, tiled layouts."""
    x = np.asarray(x, dtype=np.float32)
    weight = np.asarray(weight, dtype=np.float32)

    sx = np.float32(FP8_MAX) / np.maximum(_amax(x), np.float32(1e-12))
    sw = np.float32(FP8_MAX) / np.maximum(_amax(weight), np.float32(1e-12))
    clip = np.float32(FP8_MAX / 2.0)  # 224

    # weight: [K, N] -> per N-shard [K2, P, 2, N_CORE]:
    #   wt[k2, ki, i, n] = wq[k2*256 + i*128 + ki, nh*N_CORE + n]
    wbuf = weight * (sw * np.float32(0.5))
    np.clip(wbuf, -clip, clip, out=wbuf)
    wq = wbuf.astype(_E4M3)
    wts = [
        np.ascontiguousarray(
            wq[:, nh * N_CORE : (nh + 1) * N_CORE]
            .reshape(K2, 2, P, N_CORE)
            .transpose(0, 2, 1, 3)
        )
        for nh in range(NSHARDS)
    ]

    # x per M-shard ms: rows [ms*4096, +4096) -> [MT, P, KS, P] with
    # xt[m, ki, ks, j] = xq[m*128+j, ks*128+ki]
    x2 = x.reshape(B * S, K)
    xts = []
    for ms in range(MSHARDS):
        xbuf = x2[ms * M_CORE : (ms + 1) * M_CORE] * (sx * np.float32(0.5))
        np.clip(xbuf, -clip, clip, out=xbuf)
        xq = xbuf.astype(_E4M3)
        xts.append(np.ascontiguousarray(xq.reshape(MT, P, KS, P).transpose(0, 3, 2, 1)))

    # output scale: psum = ref_matmul / 4  ->  multiply by 4 * (1/sx) * (1/sw)
    c = np.float32(4.0) * (np.float32(1.0) / sx) * (np.float32(1.0) / sw)
    sc = np.full((P, 1), c, dtype=np.float32)
    return xts, wts, sc


def _run(x, weight, trace=False, tmpdir=None):
    from concourse.bass_utils import run_bass_kernel_spmd

    nc = _get_nc()
    xts, wts, sc = _prep(x, weight)
    in_maps = [
        {"xt": xts[c // NSHARDS], "wt": wts[c % NSHARDS], "sc": sc}
        for c in range(NCORES)
    ]
    res = run_bass_kernel_spmd(
        nc, in_maps, list(range(NCORES)), trace=trace, tmpdir=tmpdir
    )
    out = np.empty((B * S, N), dtype=np.float32)
    for c in range(NCORES):
        ms, nh = c // NSHARDS, c % NSHARDS
        out[ms * M_CORE : (ms + 1) * M_CORE, nh * N_CORE : (nh + 1) * N_CORE] = (
            res.results[c]["out"]
        )
    return out.reshape(B, S, N), res


def kernel(x, weight):
    out, _ = _run(x, weight, trace=False)
    return out


def run_traced(x, weight, tmpdir=None):
    """For test harnesses: returns (out, exec_time_ns)."""
    out, res = _run(x, weight, trace=True, tmpdir=tmpdir)
    return out, res.exec_time_ns



# revision 9
# speedup vs baseline: 1.0046x; 1.0046x over previous
"""FP8-quantized dense MLP (scaled matmul) on 8 Trainium2 NeuronCores.

Reference computation:
    x  [8, 2048, 4096] f32, weight [4096, 4096] f32
    sx = 448 / amax(|x|); sw = 448 / amax(|w|)
    out = (q8(x*sx) @ q8(w*sw)) * (1/sx) * (1/sw)     (q8 = OCP e4m3fn RNE)

Sharding: 4 M-shards x 2 N-shards over 8 cores (core c -> rows
[c//2*4096, +4096), cols [c%2*2048, +2048)).  Scales + fp8 quantization run
on host (O(MK+KN) elementwise prep); the O(MKN) matmul runs on device.

TRN2's FP8_EXP4 has max +-240 (OCP e4m3fn has +-448), so OCP-quantized values
256..448 would be NaN/Inf on device.  We therefore quantize to the OCP grid
*halved* (exact in fp8 for all but deep-subnormal values) by scaling with
sx/2 and clipping to +-224, and compensate with a *4 factor folded into the
output scale.  The device matmul (fp8 products, f32 accumulate) is then
bit-equivalent to the reference modulo f32 summation order.

Device kernel per core: out[4096, 2048] = xT.T @ w in fp8 DoubleRow mode
(K-tiles of 256).  The PE stream runs at the hardware fp8 pitch (216ns per
128x512xK256 matmul = 512 cols at the effective ~2.37GHz clock; ~442us for
the 2048 matmuls), so the schedule optimizes the edges:

- Prologue: framework preamble owns all engines until ~7.3us; the sync
  queue's first DMA bytes land ~8.7us and the early DMA pool is
  PACKET-rate limited (one packet per partition per contiguous run,
  shared 16-engine pool ramping ~25 -> ~120 pkts/us), so x0 leads the
  sync queue ahead of the 16 weight tiles (first matmul's 256 packets
  clear ~11us) while x1 rides the late-starting scalar queue.  The
  output scale is an f32 immediate baked into the eviction instructions
  (no sc DMA).  Warm-up matmuls (HAM clock ramp: 1.2GHz until ~3.4us
  busy, reset by >=1.5us idle) keep the PE hot until the first
  data-gated matmul fires, plus a batch parked between the k0 and k1
  groups where the still-ramping weight stream would otherwise stall
  the PE.
- m-tiles 0+1 run interleaved across k2 on ALL 8 PSUM banks, m1 skewed one
  k2 behind m0 (x1 arrives after w0/w1), so one arriving 524KB weight tile
  feeds 8 matmuls (~1.7us) -- a full-speed PE never outruns the weight
  stream (~1.25us/tile warm).  The warm-up tile shares bank b7 by tag with
  m1's n3 accumulator; b4-b7 are allocated only after the last warm-up
  write, because allocating a same-tag tile ROTATES the (bufs=1) slot and
  writing the rotated-out tile races the new owner (PSUM collision, device
  crash).
- m-tiles 2..30: 4 banks, alternating bank sets (b0-3 / b4-7) per m-tile,
  k2-inner; evictions (scalar/vector alternating, scaled copy) write one
  [128, 2048] SBUF tile per m-tile and a SINGLE out-DMA moves it (a DMA
  trigger costs ~0.6us of engine time; 1 trigger instead of 4).
- Last m-tile is n-outer (16 k2 matmuls per bank, then evict+DMA that bank
  immediately); banks n=1..3 evict on the vector engine (a scalar-FIFO
  ordering quirk otherwise drained n=2 last), the final bank by
  partition-half on both engines in parallel, and every tail DMA rides
  the warm sync queue (the scalar queue is packet-cold by then).
"""

import numpy as np
import ml_dtypes

FP8_MAX = 448.0
B, S, K, N = 8, 2048, 4096, 4096
NCORES = 8
MSHARDS = 4
NSHARDS = 2
M_CORE = B * S // MSHARDS   # 4096 rows per core
N_CORE = N // NSHARDS       # 2048 cols per core
P = 128
KS = K // P      # 32 k-subtiles of 128 (partition dim)
K2 = K // 256    # 16 DoubleRow k-tiles of 256
MT = M_CORE // P  # 32 m-tiles per core
NFREE = 512      # matmul free dim == one PSUM bank of f32
NT = N_CORE // NFREE  # 4 PSUM banks per m-tile

WARMUPS = 30     # PE warm-up matmuls (see _build_nc)
MID_WARMUPS = 10  # warm-ups parked between the k2=0 and k2=1 groups:
# sized so a slow-ramp run's w1 lateness is absorbed as warm-up work
# instead of a >=1.5us idle that resets the PE clock to 1.2GHz (a ~3us
# tax when it happens).  Warm-ups are unconditional PE-queue work, so
# oversizing them delays the stream when data is EARLY -- sized to the
# expected w1 gap, not the worst case.

_E4M3 = ml_dtypes.float8_e4m3  # TRN semantics: max +-240

_nc_cache = {}


def _build_nc(scale_const):
    from concourse import bacc, tile, mybir

    nc = bacc.Bacc("TRN2", debug=False)
    xt_d = nc.dram_tensor("xt", [MT, P, KS, P], mybir.dt.float8e4, kind="ExternalInput")
    wt_d = nc.dram_tensor(
        "wt", [K2, P, 2, N_CORE], mybir.dt.float8e4, kind="ExternalInput"
    )
    out_d = nc.dram_tensor("out", [M_CORE, N_CORE], mybir.dt.float32, kind="ExternalOutput")

    DR = mybir.MatmulPerfMode.DoubleRow
    KH = KS // 2   # x half-tile boundary (k2 0..7 | 8..15)
    NH = N_CORE // 2

    with tile.TileContext(nc) as tc:
        with (
            tc.tile_pool(name="wp", bufs=1) as wp,
            tc.tile_pool(name="xp", bufs=4) as xp,
            tc.tile_pool(name="op", bufs=3) as op,
            tc.tile_pool(name="fp", bufs=2) as fpool,
            tc.tile_pool(name="cp", bufs=1) as cp,
            tc.tile_pool(name="pp", bufs=1, space="PSUM") as pp,
        ):
            # PE warm-up: the HAM clock gate keeps the PE at 1.2 GHz until it
            # has been busy ~3.4us; an idle gap resets the ramp.  Dummy
            # matmuls on a zeroed scratch tile keep the PE busy from the end
            # of the framework preamble until the first data lands, so the
            # real stream starts at 2.4 GHz.  One memset only (wa serves as
            # both operands) so warm-ups start as early as possible.  The
            # warm-up accumulator shares bank b7 by tag (see module doc).
            wa = cp.tile([P, 2, P], mybir.dt.float8e4, tag="wa")
            nc.vector.memset(wa[:], 0)
            psw = pp.tile([P, P], mybir.dt.float32, tag="b7", bufs=1, name="psw")
            for _ in range(WARMUPS):
                nc.tensor.matmul(
                    psw[:], wa[:], wa[:], start=True, stop=True, perf_mode=DR
                )

            x_first = {
                m: xp.tile([P, KS, P], mybir.dt.float8e4, tag="x", name=f"x{m}")
                for m in (0, 1)
            }
            w_sb = [
                wp.tile([P, 2, N_CORE], mybir.dt.float8e4, tag=f"w{k2}",
                        name=f"w{k2}")
                for k2 in range(K2)
            ]

            # The early DMA system is PACKET-rate limited: each transfer
            # costs one packet per partition per contiguous run, and the
            # shared 16-engine pool ramps ~25/us at 8.5us -> ~90/us at 9.5us
            # -> ~120/us warm.  Measured queue behavior (ntff dma packets):
            # the sync queue's (Q1) first bytes land ~8.7us, but the scalar
            # queue (Q10) only starts flowing at ~10.2us and ramps slower.
            # The first matmul needs x0+w0 = 256 packets, so BOTH ride the
            # sync queue, x0 first: they clear the pool by ~11us (measured
            # first data matmul 11.2us, vs 14.1us when x0 rode Q10 behind
            # concurrent w1-w3 traffic).  x1 rides the otherwise-empty
            # scalar queue (lands ~12.5us, needed ~15.5us), keeping its 128
            # packets off the weight stream.  gpsimd's software-DGE queue
            # measured a 95us regression in a previous session.
            nc.sync.dma_start(x_first[0][:], xt_d[0])
            for k2 in range(K2):
                nc.sync.dma_start(w_sb[k2][:], wt_d[k2])
            nc.scalar.dma_start(x_first[1][:], xt_d[1])

            def mm_one(bank, x_t, k2, n, start, stop, load=True):
                inst = nc.tensor.matmul(
                    bank[:],
                    x_t[:, 2 * k2 : 2 * k2 + 2, :],
                    w_sb[k2][:, :, n * NFREE : (n + 1) * NFREE],
                    start=start,
                    stop=stop,
                    perf_mode=DR,
                )
                if not load:
                    # The 4 n-chunks of a k2 group share the same stationary
                    # (the x slice).  Hint codegen to skip the redundant
                    # LDWEIGHTS; measured: current walrus IGNORES this flag
                    # (still 1 LDWEIGHTS per matmul, pitch 216ns either way),
                    # kept as documentation of the attempt.
                    inst.ins.ldweights = False

            # Output scale is baked into the eviction instructions as an
            # f32 immediate (the program is compiled per scale value); this
            # removes the [128,1] sc DMA -- 128 packets off the cold early
            # pool -- and the sc_sb dependency from every eviction.
            SCALE = float(scale_const)

            def evict_mtile(m, banks):
                o_t = op.tile([P, N_CORE], mybir.dt.float32, tag="o", name=f"o{m}")
                for n in range(NT):
                    dst = o_t[:, n * NFREE : (n + 1) * NFREE]
                    if n % 2 == 0:
                        nc.scalar.activation(
                            dst,
                            banks[n][:],
                            mybir.ActivationFunctionType.Copy,
                            scale=SCALE,
                        )
                    else:
                        nc.vector.tensor_scalar_mul(dst, banks[n][:], SCALE)
                nc.sync.dma_start(out_d[m * P : (m + 1) * P, :], o_t[:])

            # ---- m-tiles 0+1, interleaved across k2 on all 8 banks ----
            # Skewed by one k2 (m1 lags m0) so m1's first matmul lands when
            # x1 -- second on the scalar queue -- has arrived; each w tile
            # still feeds 8 matmuls (~1.7us) per ~1.25us arrival, so a
            # full-speed PE never outruns the weight stream.  PSUM
            # accumulation order within a bank is k2-agnostic.
            b01 = [
                pp.tile([P, NFREE], mybir.dt.float32, tag=f"b{i}", bufs=1,
                        name=f"ps01_{i}")
                for i in range(4)
            ]
            for n in range(NT):
                mm_one(b01[n], x_first[0], 0, n, True, False, load=n == 0)
            # The k2=0 group outruns the still-ramping weight stream; park
            # the PE on warm-up matmuls instead of idling.  m1's banks b4-b7
            # are deliberately allocated only AFTER these writes: allocating
            # a same-tag tile rotates the (bufs=1) slot, and writing the
            # rotated-out psw tile afterwards races the new owner's
            # accumulation (PSUM_COLLISION device crash).
            for _ in range(MID_WARMUPS):
                nc.tensor.matmul(
                    psw[:], wa[:], wa[:], start=True, stop=True, perf_mode=DR
                )
            b01 += [
                pp.tile([P, NFREE], mybir.dt.float32, tag=f"b{i}", bufs=1,
                        name=f"ps01_{i}")
                for i in range(4, 8)
            ]
            # m1 lags m0 by SKEW k2-steps: blocks k1..k(SKEW-1) are m0-only
            # (0.85us demand per w tile), so a lagging cold-ramp weight
            # stream produces several sub-1.5us stalls (no clock reset)
            # instead of one big resetting one; m1 repays the lag in
            # supply-free tail blocks.  m0 finishes first, so its eviction
            # overlaps m1's tail.
            SKEW = 4
            for k2 in range(1, K2):
                for n in range(NT):
                    mm_one(b01[n], x_first[0], k2, n, False, k2 == K2 - 1,
                           load=n == 0)
                if k2 >= SKEW:
                    j = k2 - SKEW
                    for n in range(NT):
                        mm_one(b01[4 + n], x_first[1], j, n, j == 0, False,
                               load=n == 0)
            evict_mtile(0, b01[0:4])
            for j in range(K2 - SKEW, K2):
                for n in range(NT):
                    mm_one(b01[4 + n], x_first[1], j, n, False, j == K2 - 1,
                           load=n == 0)
            evict_mtile(1, b01[4:8])

            # ---- m-tiles 2..30: 4 banks, alternating sets, k2-inner ----
            for m in range(2, MT - 1):
                x_t = xp.tile([P, KS, P], mybir.dt.float8e4, tag="x", name=f"x{m}")
                nc.sync.dma_start(x_t[:], xt_d[m])
                base = (m % 2) * 4
                banks = [
                    pp.tile([P, NFREE], mybir.dt.float32, tag=f"b{base + n}",
                            bufs=1, name=f"ps{m}_{n}")
                    for n in range(NT)
                ]
                for k2 in range(K2):
                    for n in range(NT):
                        mm_one(banks[n], x_t, k2, n, k2 == 0, k2 == K2 - 1,
                               load=n == 0)
                evict_mtile(m, banks)

            # ---- last m-tile: n-outer so the tail is one bank deep.
            # Evictions n=1..3 all run on the VECTOR engine: with n=2 on
            # scalar, the Tile scheduler ordered it AFTER n=3's half-evict
            # in the scalar FIFO, so n=2's 256KB out-DMA became the LAST to
            # drain (+2us of tail).  Every tail out-DMA rides the SYNC
            # queue: the scalar queue has been idle since ~15us and is
            # packet-cold at the end (measured 32 pkts/us vs 120 warm).
            # Each eviction gets a unique tile tag so no slot-rotation
            # ordering is induced between them.
            m = MT - 1
            x_t = xp.tile([P, KS, P], mybir.dt.float8e4, tag="x", name=f"x{m}")
            nc.sync.dma_start(x_t[:], xt_d[m])
            for n in range(NT):
                bank = pp.tile([P, NFREE], mybir.dt.float32, tag=f"b{4 + n}",
                               bufs=1, name=f"ps{m}_{n}")
                for k2 in range(K2):
                    mm_one(bank, x_t, k2, n, k2 == 0, k2 == K2 - 1)
                if n < NT - 1:
                    o_t = fpool.tile([P, NFREE], mybir.dt.float32, tag=f"of{n}",
                                     name=f"o{m}_{n}")
                    if n == 0:
                        nc.scalar.activation(
                            o_t[:], bank[:], mybir.ActivationFunctionType.Copy,
                            scale=SCALE,
                        )
                    else:
                        nc.vector.tensor_scalar_mul(o_t[:], bank[:], SCALE)
                    nc.sync.dma_start(
                        out_d[m * P : (m + 1) * P,
                              n * NFREE : (n + 1) * NFREE],
                        o_t[:],
                    )
                else:
                    # final bank: evict by partition-half on both engines and
                    # launch each half's DMA as soon as its half lands.  The
                    # halves go to SEPARATE tiles: sharing one tile serialized
                    # them (subtile hazard tracking does not split partition
                    # ranges -- measured: the vector copy waited on the
                    # scalar copy's completion semaphore).
                    hp = P // 2
                    o_lo = fpool.tile([P, NFREE], mybir.dt.float32,
                                      tag="oflo", name=f"o{m}_{n}lo")
                    o_hi = fpool.tile([P, NFREE], mybir.dt.float32,
                                      tag="ofhi", name=f"o{m}_{n}hi")
                    nc.scalar.activation(
                        o_lo[0:hp, :], bank[0:hp, :],
                        mybir.ActivationFunctionType.Copy,
                        scale=SCALE,
                    )
                    nc.sync.dma_start(
                        out_d[m * P : m * P + hp,
                              n * NFREE : (n + 1) * NFREE],
                        o_lo[0:hp, :],
                    )
                    nc.vector.tensor_scalar_mul(
                        o_hi[hp:, :], bank[hp:, :], SCALE
                    )
                    nc.sync.dma_start(
                        out_d[m * P + hp : (m + 1) * P,
                              n * NFREE : (n + 1) * NFREE],
                        o_hi[hp:, :],
                    )

    nc.finalize()
    return nc


def _get_nc(scale_const):
    key = float(scale_const)
    if key not in _nc_cache:
        _nc_cache[key] = _build_nc(key)
    return _nc_cache[key]


def _amax(a):
    # max(|a|) without a full |a| temp; exact (max/min are exact in f32)
    return np.float32(max(np.float32(a.max()), -np.float32(a.min())))


def _prep(x, weight):
    """Host prep: scales, halved OCP-grid fp8 quantization, tiled layouts."""
    x = np.asarray(x, dtype=np.float32)
    weight = np.asarray(weight, dtype=np.float32)

    sx = np.float32(FP8_MAX) / np.maximum(_amax(x), np.float32(1e-12))
    sw = np.float32(FP8_MAX) / np.maximum(_amax(weight), np.float32(1e-12))
    clip = np.float32(FP8_MAX / 2.0)  # 224

    # weight: [K, N] -> per N-shard [K2, P, 2, N_CORE]:
    #   wt[k2, ki, i, n] = wq[k2*256 + i*128 + ki, nh*N_CORE + n]
    wbuf = weight * (sw * np.float32(0.5))
    np.clip(wbuf, -clip, clip, out=wbuf)
    wq = wbuf.astype(_E4M3)
    wts = [
        np.ascontiguousarray(
            wq[:, nh * N_CORE : (nh + 1) * N_CORE]
            .reshape(K2, 2, P, N_CORE)
            .transpose(0, 2, 1, 3)
        )
        for nh in range(NSHARDS)
    ]

    # x per M-shard ms: rows [ms*4096, +4096) -> [MT, P, KS, P] with
    # xt[m, ki, ks, j] = xq[m*128+j, ks*128+ki]
    x2 = x.reshape(B * S, K)
    xts = []
    for ms in range(MSHARDS):
        xbuf = x2[ms * M_CORE : (ms + 1) * M_CORE] * (sx * np.float32(0.5))
        np.clip(xbuf, -clip, clip, out=xbuf)
        xq = xbuf.astype(_E4M3)
        xts.append(np.ascontiguousarray(xq.reshape(MT, P, KS, P).transpose(0, 3, 2, 1)))

    # output scale: psum = ref_matmul / 4  ->  multiply by 4 * (1/sx) * (1/sw)
    c = np.float32(4.0) * (np.float32(1.0) / sx) * (np.float32(1.0) / sw)
    return xts, wts, c


def _run(x, weight, trace=False, tmpdir=None):
    from concourse.bass_utils import run_bass_kernel_spmd

    xts, wts, sc = _prep(x, weight)
    nc = _get_nc(sc)
    in_maps = [
        {"xt": xts[c // NSHARDS], "wt": wts[c % NSHARDS]}
        for c in range(NCORES)
    ]
    res = run_bass_kernel_spmd(
        nc, in_maps, list(range(NCORES)), trace=trace, tmpdir=tmpdir
    )
    out = np.empty((B * S, N), dtype=np.float32)
    for c in range(NCORES):
        ms, nh = c // NSHARDS, c % NSHARDS
        out[ms * M_CORE : (ms + 1) * M_CORE, nh * N_CORE : (nh + 1) * N_CORE] = (
            res.results[c]["out"]
        )
    return out.reshape(B, S, N), res


def kernel(x, weight):
    out, _ = _run(x, weight, trace=False)
    return out


def run_traced(x, weight, tmpdir=None):
    """For test harnesses: returns (out, exec_time_ns)."""
    out, res = _run(x, weight, trace=True, tmpdir=tmpdir)
    return out, res.exec_time_ns

